# revision 48
# baseline (speedup 1.0000x reference)
"""SAGAN self-attention block on 8 TRN2 NeuronCores.

Reference (per batch element b, N = H*W = 4096, C = 512, D = 64):
    f = x @ Wf + bf ; g = x @ Wg + bg ; h = x @ Wh + bh      # [N, D]
    s = f @ g.T                                              # [N, N]
    attn = softmax(s, axis=-1)
    ctx = attn @ h                                           # [N, D]
    o = (gamma * ctx) @ Wv + bv + x                          # [N, C]

Sharding: data-parallel over batch B=8 -> one batch element per core, no
collectives. Weights replicated.

Device algorithm (per core), matmuls in bf16 with f32 PSUM accumulation:
  - x arrives twice: f32 (residual) and as a host-side bf16 transposed
    copy xbt [C, N] (input layout prep, like the weight packing) -- no
    on-device transposes of x at all, and few, large, contiguous DMAs.
  - f and g projected in one matmul chain per 512-chunk using stacked
    [Wf|Wg] weights (M=128); each half is mirrored into the other
    partition half of FT2/GT2 via SBUF->SBUF DMA so QK pairs row-pack.
  - haug[m, :] = [x@Wh + bh, 1.0] -> [4096, 65] bf16 (m on partitions).
  - unnormalized softmax (no max subtraction: max s ~ 67, e^67 fits bf16):
      for each n-chunk of 512 columns:
        for each pair (i0, i1) of 128-row m-tiles:
          S'[i0]|S'[i1] computed concurrently via K=64 row-packing into
          one [128, 1024] f32 PSUM tile
          E' = exp(S') -> bf16, produced by ONE of:
            * ScalarE ACT exp (exact), or
            * VectorE Schraudolph bit-trick: bf16_bits = i16(s*184.665
              + 16248.5) ~= exp(s) to ~2% rms -- one tensor_scalar op.
          The split keeps both engines busy; softmax ratios stay
          consistent because numerator and denominator use the same E'.
          ctxT[0:65, chunk] += haug[i0].T @ E'[:, :512] + haug[i1].T @ ...
        row 64 of ctxT = sum_m E' = softmax denominator (ones column)
  - epilogue per chunk: denominators for all 4 row-tiles are gathered by
    one tiny DMA + one PE transpose; out = (ctxT.T @ [gamma*Wv ; bv])
    * (1/denom) + x, residual add on GpSimd.
  - prologue (loads, transposes, f/g/h projections) is emitted
    interleaved with attention chunk 0 so exp work starts immediately.
"""

import numpy as np
import ml_dtypes

BF16 = ml_dtypes.bfloat16

B, HH, WW, C = 8, 64, 64, 512
D = C // 8          # 64
N_FULL = HH * WW    # 4096
P = 128
CC = C // P         # 4  (c-chunks of 128)

# Schraudolph bf16 exp: bits = round(x * 184.665 + 16248.5), view as bf16.
SCH_A = 128.0 / float(np.log(2.0))
SCH_B = 16248.5
# Pairs whose exp runs on VectorE via the bit trick (rest on ScalarE).
# Pairs advance in bursts of two (2k, 2k+1); putting the VectorE exp on the
# odd member makes each burst's two exps run on different engines
# concurrently, halving the sp PSUM-slot recycle latency. Chunk 0 runs
# during the prologue, where VectorE is busy with h-copies -- keep its
# VectorE share small and late.
SCH_PAIRS = frozenset({1, 7, 9, 11, 13, 15})
SCH_PAIRS_C0 = frozenset({5, 10, 13, 15})

_CACHE: dict = {}


def _build(n: int, h_bias_zero: bool = False):
    """Build + compile the single-core Bass program (same NEFF on all 8 cores)."""
    import concourse.mybir as mybir
    from concourse import bacc
    from concourse.tile import TileContext

    f32 = mybir.dt.float32
    bf16 = mybir.dt.bfloat16
    i16 = mybir.dt.int16
    ADD = mybir.AluOpType.add
    MULT = mybir.AluOpType.mult
    EXP = mybir.ActivationFunctionType.Exp

    n_tiles = n // P          # 32
    n_pairs = n_tiles // 2    # 16
    nch = n // 512            # 8

    nc = bacc.Bacc("TRN2", target_bir_lowering=False, debug=False)

    x_d = nc.dram_tensor("x", [n, C], f32, kind="ExternalInput")
    xbt_d = nc.dram_tensor("xbt", [C, n], bf16, kind="ExternalInput")
    # two stacked projection blocks: [Wf|Wg] and [Wg|Wf] -- the swapped copy
    # lets both partition-halves of FT2/GT2 come straight from matmuls,
    # with no cross-partition mirror DMAs.
    wfg_d = nc.dram_tensor("wfg", [C, 4 * D], bf16, kind="ExternalInput")
    wh_d = nc.dram_tensor("wh", [C, D], bf16, kind="ExternalInput")
    bfg_d = nc.dram_tensor("bfg", [P, 2], f32, kind="ExternalInput")   # [bf;bg],[bg;bf]
    if not h_bias_zero:
        bh_d = nc.dram_tensor("bhp", [1, D], bf16, kind="ExternalInput")
        on_d = nc.dram_tensor("onesp", [1, P], bf16, kind="ExternalInput")
    wv_d = nc.dram_tensor("wv", [D + 1, C], bf16, kind="ExternalInput")
    id_d = nc.dram_tensor("ident", [P, P], bf16, kind="ExternalInput")
    out_d = nc.dram_tensor("out", [n, C], f32, kind="ExternalOutput")

    # big-DMA views: one load per 512-row block
    x_blk = x_d.rearrange("(ni t p) c -> ni p t c", p=P, t=4)
    xbt_blk = xbt_d.rearrange("(cc p) n -> p cc n", p=P)
    o_t = out_d.rearrange("(i p) c -> i p c", p=P)

    with TileContext(nc) as tc:
        with (
            tc.tile_pool(name="const", bufs=1) as cpool,
            tc.tile_pool(name="big", bufs=1) as bigpool,
            tc.tile_pool(name="ep", bufs=4) as epool,
            tc.tile_pool(name="ct", bufs=2) as ctpool,
            tc.tile_pool(name="os", bufs=4) as opool,
            tc.tile_pool(name="sm", bufs=4) as smpool,
            tc.tile_pool(name="psA", bufs=3, space="PSUM") as psA,
            tc.tile_pool(name="psB", bufs=2, space="PSUM") as psB,
        ):
            # ---- replicated constants -> SBUF
            wfg_sb = cpool.tile([P, CC, 4 * D], bf16)
            nc.sync.dma_start(wfg_sb, wfg_d.rearrange("(cc p) d -> p cc d", p=P))
            wh_sb = cpool.tile([P, CC, D], bf16)
            nc.sync.dma_start(wh_sb, wh_d.rearrange("(cc p) d -> p cc d", p=P))
            bfg_sb = cpool.tile([P, 2], f32)
            nc.sync.dma_start(bfg_sb, bfg_d[:, :])
            if not h_bias_zero:
                bh_sb = cpool.tile([1, D], bf16)
                nc.sync.dma_start(bh_sb, bh_d[:, :])
                ones_sb = cpool.tile([1, P], bf16)
                nc.sync.dma_start(ones_sb, on_d[:, :])
            zbias = cpool.tile([P, 1], f32)
            nc.gpsimd.memset(zbias, 0.0)

            # warm the ACT exp table while the prologue DMAs run
            warm = cpool.tile([P, 1], bf16)
            nc.scalar.activation(warm, bfg_sb[:, 0:1], EXP,
                                 bias=zbias[:, 0:1], scale=0.0)

            # ---- persistent SBUF tensors
            xres = bigpool.tile([P, n_tiles, C], f32)    # x rows (residual)
            xT = bigpool.tile([P, CC, n], bf16)          # x transposed (c on partitions)
            FT2 = bigpool.tile([P, n], bf16)             # f.T duplicated in both halves
            GT2 = bigpool.tile([P, n], bf16)             # g.T duplicated in both halves
            haug = bigpool.tile([P, n_tiles, D + 1], bf16)
            nc.gpsimd.memset(haug[:, :, D:D + 1], 1.0)

            IDEN = mybir.ActivationFunctionType.Identity

            def emit_proj_block(ni):
                """Two f/g chains ([Wf|Wg] and [Wg|Wf]) + four h chains for
                x-tiles 4ni..4ni+3.

                Chained PSUM accumulation exposes the PE drain latency, so
                independent chains are interleaved (ring depth 3) to keep
                back-to-back matmuls on different PSUM banks.
                """
                sl = slice(ni * 512, (ni + 1) * 512)
                fgA = psA.tile([P, 512], f32, tag="sp", name=f"fgA{ni}")
                fgB = psA.tile([P, 512], f32, tag="sp", name=f"fgB{ni}")
                hp = [psA.tile([P, D], f32, tag="sp", name=f"hps{4 * ni + t}")
                      for t in range(4)]

                def fg_mm(b, cc):
                    nc.tensor.matmul(
                        fgA if b == 0 else fgB,
                        lhsT=wfg_sb[:, cc, 2 * D * b:2 * D * (b + 1)],
                        rhs=xT[:, cc, sl],
                        start=(cc == 0), stop=(cc == CC - 1),
                    )

                def h_mm(t, cc):
                    i = 4 * ni + t
                    nc.tensor.matmul(
                        hp[t], lhsT=xT[:, cc, i * P:(i + 1) * P],
                        rhs=wh_sb[:, cc, :],
                        start=(cc == 0), stop=(h_bias_zero and cc == CC - 1),
                    )

                # group 1: fgA + fgB + h0 round-robin (3 live PSUM tiles)
                for cc in range(CC):
                    fg_mm(0, cc)
                    fg_mm(1, cc)
                    h_mm(0, cc)
                # A: [f;g] -> FT2 top, GT2 bottom; B: [g;f] -> GT2 top, FT2 bottom
                # (copy+bias+cast split across ScalarE and VectorE -- scalar
                # Copy-class ops pay the 2.3x errata, ~1.3us each)
                nc.vector.tensor_scalar(
                    FT2[0:D, sl], fgA[0:D, :], bfg_sb[0:D, 0:1], None, ADD)
                nc.scalar.activation(GT2[D:P, sl], fgA[D:P, :], IDEN,
                                     bias=bfg_sb[D:P, 0:1], scale=1.0)
                nc.vector.tensor_scalar(
                    GT2[0:D, sl], fgB[0:D, :], bfg_sb[0:D, 1:2], None, ADD)
                nc.scalar.activation(FT2[D:P, sl], fgB[D:P, :], IDEN,
                                     bias=bfg_sb[D:P, 1:2], scale=1.0)
                # group 2: h1 + h2 + h3
                for cc in range(CC):
                    h_mm(1, cc)
                    h_mm(2, cc)
                    h_mm(3, cc)
                for t in range(4):
                    i = 4 * ni + t
                    if not h_bias_zero:
                        nc.tensor.matmul(
                            hp[t], lhsT=ones_sb, rhs=bh_sb, start=False, stop=True)
                    nc.vector.tensor_copy(out=haug[:, i, 0:D], in_=hp[t])

            def emit_qk_exp(jc, ip):
                sl = slice(jc * 512, (jc + 1) * 512)
                i0, i1 = 2 * ip, 2 * ip + 1
                sp = psA.tile([P, 1024], f32, tag="sp", name=f"sp{jc}_{ip}")
                nc.tensor.matmul(
                    sp[:, 0:512],
                    lhsT=GT2[0:D, i0 * P:(i0 + 1) * P], rhs=FT2[0:D, sl],
                    start=True, stop=True, tile_position=(0, 0),
                )
                nc.tensor.matmul(
                    sp[:, 512:1024],
                    lhsT=GT2[D:P, i1 * P:(i1 + 1) * P], rhs=FT2[D:P, sl],
                    start=True, stop=True, tile_position=(D, 0),
                )
                ep = epool.tile([P, 1024], bf16, tag="ep", name=f"ep{jc}_{ip}")
                if ip in (SCH_PAIRS_C0 if jc == 0 else SCH_PAIRS):
                    # VectorE Schraudolph exp: one tensor_scalar into bf16 bits
                    nc.vector.tensor_scalar(
                        ep[:].bitcast(i16), sp, SCH_A, SCH_B, MULT, ADD)
                else:
                    nc.scalar.activation(ep, sp, EXP, bias=zbias[:, 0:1], scale=1.0)
                ep_tiles[(jc, ip)] = ep

            def emit_pv(jc, ip):
                i0, i1 = 2 * ip, 2 * ip + 1
                ep = ep_tiles.pop((jc, ip))
                ctx = ctx_tiles[jc]
                nc.tensor.matmul(
                    ctx, lhsT=haug[:, i0, :], rhs=ep[:, 0:512],
                    start=(ip == 0), stop=False,
                )
                nc.tensor.matmul(
                    ctx, lhsT=haug[:, i1, :], rhs=ep[:, 512:1024],
                    start=False, stop=(ip == n_pairs - 1),
                )

            def chunk_epilogue_a(jc):
                # latency part: PSUM evacuation + denominator gather DMAs.
                # Runs early so the small-DMA latency hides under attention.
                ctx = ctx_tiles[jc]
                ct = ctpool.tile([D + 1, 512], bf16, tag="ct", name=f"ct{jc}")
                nc.vector.tensor_copy(out=ct, in_=ctx)
                # gather the 4 subtile denominators: [1, 512] -> [4, 128] -> [128, 4]
                dn4 = smpool.tile([4, P], bf16, tag="dn", name=f"dn{jc}")
                # the last chunk's gather must not queue behind stores: use
                # the (by then idle) scalar HWDGE queue
                eng = nc.scalar if jc == nch - 1 else nc.sync
                for t in range(4):
                    eng.dma_start(dn4[t:t + 1, :], ct[D:D + 1, t * P:(t + 1) * P])
                ct_tiles[jc] = (ct, dn4)

            def epilogue_units(jc):
                """The chunk epilogue as 5 small units, interleaved between
                attention steps so its matmuls/DVE ops never bunch up."""
                ct, dn4 = ct_tiles.pop(jc)
                rc = smpool.tile([P, 4], f32, tag="rc", name=f"rc{jc}")

                def u_denom():
                    dtp = psA.tile([P, 4], bf16, tag="sp", name=f"dtp{jc}")
                    nc.tensor.transpose(dtp, dn4, id_sb[0:4, 0:4])
                    rce = smpool.tile([P, 4], f32, tag="rce", name=f"rce{jc}")
                    nc.vector.tensor_scalar(rce, dtp, 1e-30, None, ADD)
                    nc.vector.reciprocal(rc, rce)
                yield u_denom

                def u_out(t):
                    it = jc * 4 + t
                    tsl = slice(t * P, (t + 1) * P)
                    op = psA.tile([P, C], f32, tag="sp", name=f"op{it}")
                    nc.tensor.matmul(op, lhsT=ct[:, tsl], rhs=wv_sb, start=True, stop=True)
                    osb = opool.tile([P, C], f32, tag="os", name=f"osb{it}")
                    nc.vector.scalar_tensor_tensor(
                        osb, op, rc[:, t:t + 1], xres[:, it, :], MULT, ADD)
                    if jc == nch - 1:
                        # final chunk's stores are the kernel tail: split across
                        # two DMAs to halve the drain latency
                        nc.sync.dma_start(o_t[it][0:D, :], osb[0:D, :])
                        nc.scalar.dma_start(o_t[it][D:P, :], osb[D:P, :])
                    else:
                        nc.sync.dma_start(o_t[it], osb)
                for t in range(4):
                    yield (lambda tt: (lambda: u_out(tt)))(t)

            # ---- software-pipelined emission over all (chunk, pair) steps:
            # QK+exp run LOOKAHEAD pairs ahead of PV so the PE queue never
            # stalls waiting for an exp; epilogues trail by EPI_DELAY pairs.
            ctx_tiles = {}
            for jc in range(nch):
                ctx_tiles[jc] = psB.tile([D + 1, 512], f32, tag="ctx", name=f"ctx{jc}")
            ep_tiles = {}
            ct_tiles = {}
            pairs = [(jc, ip) for jc in range(nch) for ip in range(n_pairs)]
            LOOKAHEAD = 2
            EPI_DELAY_A = 1
            EPI_DELAY = 10
            n_total = len(pairs)
            qk_done = 0
            pv_done = 0
            epia_done = 0
            epi_done = 0

            xres_done = 0
            epi_queue = []

            def advance_pv():
                nonlocal pv_done, epia_done, epi_done, xres_done
                emit_pv(*pairs[pv_done])
                pv_done += 1
                # paced residual loads: issued as attention progresses so
                # they never crowd out the critical xT loads on sync
                if xres_done < nch and pv_done >= 16 + 2 * xres_done:
                    nc.sync.dma_start(
                        xres[:, 4 * xres_done:4 * xres_done + 4, :],
                        x_blk[xres_done])
                    xres_done += 1
                while epia_done < nch and pv_done >= min(
                        (epia_done + 1) * n_pairs + EPI_DELAY_A, n_total):
                    chunk_epilogue_a(epia_done)
                    epia_done += 1
                while epi_done < nch and pv_done >= min(
                        (epi_done + 1) * n_pairs + EPI_DELAY, n_total):
                    epi_queue.extend(epilogue_units(epi_done))
                    epi_done += 1

            # prologue: per 512-row block, load + project, then advance the
            # attention pipeline through chunk 0's pairs as they become legal
            wv_sb = id_sb = None
            for ni in range(nch):
                nsl = slice(ni * 512, (ni + 1) * 512)
                # xT loads own the sync queue: they gate f/g/h and chunk 0
                nc.sync.dma_start(xT[:, :, nsl], xbt_blk[:, :, nsl])
                if ni == 1:
                    # epilogue-only constants: loaded after the critical path
                    wv_sb = cpool.tile([D + 1, C], bf16)
                    nc.sync.dma_start(wv_sb, wv_d[:, :])
                    id_sb = cpool.tile([P, P], bf16)
                    nc.sync.dma_start(id_sb, id_d[:, :])
                emit_proj_block(ni)
                while qk_done <= 2 * ni + 1:
                    emit_qk_exp(*pairs[qk_done])
                    qk_done += 1
                while pv_done < qk_done - LOOKAHEAD:
                    advance_pv()

            # steady state: advance two pairs per step -- batching the two
            # QKs then the two PVs halves the QK<->PV row-group transitions
            # (each costs ~100ns of LDWEIGHTS that can't overlap in-flight
            # matmuls on conflicting PE rows)
            while pv_done < n_total:
                for _ in range(2):
                    if qk_done < n_total:
                        emit_qk_exp(*pairs[qk_done])
                        qk_done += 1
                while pv_done < min(qk_done - LOOKAHEAD, n_total):
                    advance_pv()
                # one epilogue unit per step keeps its matmuls spread out
                if epi_queue:
                    epi_queue.pop(0)()
                if qk_done == n_total:
                    while pv_done < n_total:
                        advance_pv()
            while epi_queue:
                epi_queue.pop(0)()

    nc.compile()
    return nc


def get_program(n: int = N_FULL, h_bias_zero: bool = False):
    key = (n, h_bias_zero)
    if key not in _CACHE:
        _CACHE[key] = _build(n, h_bias_zero)
    return _CACHE[key]


def make_weight_maps(Wf, bf, Wg, bg, Wh, bh, Wv, bv, gamma, h_bias_zero=False):
    """Host-side layout prep of the tiny replicated weights."""
    wv_aug = np.concatenate(
        [np.float32(gamma) * np.asarray(Wv, np.float32),
         np.asarray(bv, np.float32)[None, :]], axis=0)
    bf = np.asarray(bf, np.float32)
    bg = np.asarray(bg, np.float32)
    bfg = np.stack([np.concatenate([bf, bg]), np.concatenate([bg, bf])], axis=1)
    Wf = np.asarray(Wf, np.float32)
    Wg = np.asarray(Wg, np.float32)
    wfg = np.concatenate([Wf, Wg, Wg, Wf], axis=1)
    maps = {
        "wfg": np.ascontiguousarray(wfg.astype(BF16)),
        "wh": np.ascontiguousarray(np.asarray(Wh, np.float32).astype(BF16)),
        "bfg": np.ascontiguousarray(bfg),
        "bhp": np.ascontiguousarray(np.asarray(bh, np.float32).astype(BF16).reshape(1, D)),
        "onesp": np.ones((1, P), dtype=BF16),
        "wv": np.ascontiguousarray(wv_aug.astype(BF16)),
        "ident": np.ascontiguousarray(np.eye(P, dtype=BF16)),
    }
    if h_bias_zero:
        del maps["bhp"], maps["onesp"]
    return maps


def kernel(x, Wf, bf, Wg, bg, Wh, bh, Wv, bv, gamma):
    from concourse.bass_utils import run_bass_kernel_spmd

    x = np.asarray(x, np.float32)
    b, hh, ww, c = x.shape
    n = hh * ww
    assert (b, c) == (B, C)

    hbz = bool(np.all(np.asarray(bh) == 0))
    nc = get_program(n, hbz)
    base = make_weight_maps(Wf, bf, Wg, bg, Wh, bh, Wv, bv, gamma, hbz)
    xf = x.reshape(b, n, c)
    in_maps = [
        dict(base,
             x=np.ascontiguousarray(xf[i]),
             xbt=np.ascontiguousarray(xf[i].T.astype(BF16)))
        for i in range(b)
    ]

    res = run_bass_kernel_spmd(nc, in_maps, core_ids=list(range(b)))
    out = np.stack([res.results[i]["out"] for i in range(b)], axis=0)
    return np.ascontiguousarray(out.reshape(b, hh, ww, c).astype(np.float32))


# revision 49
# speedup vs baseline: 1.0151x; 1.0151x over previous
"""SAGAN self-attention block on 8 TRN2 NeuronCores.

Reference (per batch element b, N = H*W = 4096, C = 512, D = 64):
    f = x @ Wf + bf ; g = x @ Wg + bg ; h = x @ Wh + bh      # [N, D]
    s = f @ g.T                                              # [N, N]
    attn = softmax(s, axis=-1)
    ctx = attn @ h                                           # [N, D]
    o = (gamma * ctx) @ Wv + bv + x                          # [N, C]

Sharding: data-parallel over batch B=8 -> one batch element per core, no
collectives. Weights replicated.

Device algorithm (per core), matmuls in bf16 with f32 PSUM accumulation:
  - x arrives twice: f32 (residual) and as a host-side bf16 transposed
    copy xbt [C, N] (input layout prep, like the weight packing) -- no
    on-device transposes of x at all, and few, large, contiguous DMAs.
  - f and g projected in one matmul chain per 512-chunk using stacked
    [Wf|Wg] weights (M=128); each half is mirrored into the other
    partition half of FT2/GT2 via SBUF->SBUF DMA so QK pairs row-pack.
  - haug[m, :] = [x@Wh + bh, 1.0] -> [4096, 65] bf16 (m on partitions).
  - unnormalized softmax (no max subtraction: max s ~ 67, e^67 fits bf16):
      for each n-chunk of 512 columns:
        for each pair (i0, i1) of 128-row m-tiles:
          S'[i0]|S'[i1] computed concurrently via K=64 row-packing into
          one [128, 1024] f32 PSUM tile
          E' = exp(S') -> bf16, produced by ONE of:
            * ScalarE ACT exp (exact), or
            * VectorE Schraudolph bit-trick: bf16_bits = i16(s*184.665
              + 16248.5) ~= exp(s) to ~2% rms -- one tensor_scalar op.
          The split keeps both engines busy; softmax ratios stay
          consistent because numerator and denominator use the same E'.
          ctxT[0:65, chunk] += haug[i0].T @ E'[:, :512] + haug[i1].T @ ...
        row 64 of ctxT = sum_m E' = softmax denominator (ones column)
  - epilogue per chunk: denominators for all 4 row-tiles are gathered by
    one tiny DMA + one PE transpose; out = (ctxT.T @ [gamma*Wv ; bv])
    * (1/denom) + x, residual add on GpSimd.
  - prologue (loads, transposes, f/g/h projections) is emitted
    interleaved with attention chunk 0 so exp work starts immediately.
"""

import numpy as np
import ml_dtypes

BF16 = ml_dtypes.bfloat16

B, HH, WW, C = 8, 64, 64, 512
D = C // 8          # 64
N_FULL = HH * WW    # 4096
P = 128
CC = C // P         # 4  (c-chunks of 128)

# Schraudolph bf16 exp: bits = round(x * 184.665 + 16248.5), view as bf16.
SCH_A = 128.0 / float(np.log(2.0))
SCH_B = 16248.5
# Pairs whose exp runs on VectorE via the bit trick (rest on ScalarE).
# Pairs advance in bursts of two (2k, 2k+1); putting the VectorE exp on the
# odd member makes each burst's two exps run on different engines
# concurrently, halving the sp PSUM-slot recycle latency. Chunk 0 runs
# during the prologue, where VectorE is busy with h-copies -- keep its
# VectorE share small and late.
SCH_PAIRS = frozenset({2, 5, 7, 10, 13, 15})
SCH_PAIRS_C0 = SCH_PAIRS

_CACHE: dict = {}


def _build(n: int, h_bias_zero: bool = False):
    """Build + compile the single-core Bass program (same NEFF on all 8 cores)."""
    import concourse.mybir as mybir
    from concourse import bacc
    from concourse.tile import TileContext

    f32 = mybir.dt.float32
    bf16 = mybir.dt.bfloat16
    i16 = mybir.dt.int16
    ADD = mybir.AluOpType.add
    MULT = mybir.AluOpType.mult
    EXP = mybir.ActivationFunctionType.Exp

    n_tiles = n // P          # 32
    n_pairs = n_tiles // 2    # 16
    nch = n // 512            # 8

    nc = bacc.Bacc("TRN2", target_bir_lowering=False, debug=False)

    x_d = nc.dram_tensor("x", [n, C], f32, kind="ExternalInput")
    xbt_d = nc.dram_tensor("xbt", [C, n], bf16, kind="ExternalInput")
    # two stacked projection blocks: [Wf|Wg] and [Wg|Wf] -- the swapped copy
    # lets both partition-halves of FT2/GT2 come straight from matmuls,
    # with no cross-partition mirror DMAs.
    wfg_d = nc.dram_tensor("wfg", [C, 4 * D], bf16, kind="ExternalInput")
    wh_d = nc.dram_tensor("wh", [C, D], bf16, kind="ExternalInput")
    bfg_d = nc.dram_tensor("bfg", [P, 2], f32, kind="ExternalInput")   # [bf;bg],[bg;bf]
    if not h_bias_zero:
        bh_d = nc.dram_tensor("bhp", [1, D], bf16, kind="ExternalInput")
        on_d = nc.dram_tensor("onesp", [1, P], bf16, kind="ExternalInput")
    wv_d = nc.dram_tensor("wv", [D + 1, C], bf16, kind="ExternalInput")
    id_d = nc.dram_tensor("ident", [P, P], bf16, kind="ExternalInput")
    out_d = nc.dram_tensor("out", [n, C], f32, kind="ExternalOutput")

    # big-DMA views: one load per 512-row block
    x_blk = x_d.rearrange("(ni t p) c -> ni p t c", p=P, t=4)
    xbt_blk = xbt_d.rearrange("(cc p) n -> p cc n", p=P)
    o_t = out_d.rearrange("(i p) c -> i p c", p=P)

    with TileContext(nc) as tc:
        with (
            tc.tile_pool(name="const", bufs=1) as cpool,
            tc.tile_pool(name="big", bufs=1) as bigpool,
            tc.tile_pool(name="ep", bufs=4) as epool,
            tc.tile_pool(name="ct", bufs=2) as ctpool,
            tc.tile_pool(name="os", bufs=4) as opool,
            tc.tile_pool(name="sm", bufs=4) as smpool,
            tc.tile_pool(name="psA", bufs=3, space="PSUM") as psA,
            tc.tile_pool(name="psB", bufs=2, space="PSUM") as psB,
        ):
            # ---- replicated constants -> SBUF
            wfg_sb = cpool.tile([P, CC, 4 * D], bf16)
            nc.sync.dma_start(wfg_sb, wfg_d.rearrange("(cc p) d -> p cc d", p=P))
            wh_sb = cpool.tile([P, CC, D], bf16)
            nc.sync.dma_start(wh_sb, wh_d.rearrange("(cc p) d -> p cc d", p=P))
            bfg_sb = cpool.tile([P, 2], f32)
            nc.sync.dma_start(bfg_sb, bfg_d[:, :])
            if not h_bias_zero:
                bh_sb = cpool.tile([1, D], bf16)
                nc.sync.dma_start(bh_sb, bh_d[:, :])
                ones_sb = cpool.tile([1, P], bf16)
                nc.sync.dma_start(ones_sb, on_d[:, :])
            zbias = cpool.tile([P, 1], f32)
            nc.gpsimd.memset(zbias, 0.0)

            # warm the ACT exp table while the prologue DMAs run
            warm = cpool.tile([P, 1], bf16)
            nc.scalar.activation(warm, bfg_sb[:, 0:1], EXP,
                                 bias=zbias[:, 0:1], scale=0.0)

            # ---- persistent SBUF tensors
            xres = bigpool.tile([P, n_tiles, C], f32)    # x rows (residual)
            xT = bigpool.tile([P, CC, n], bf16)          # x transposed (c on partitions)
            FT2 = bigpool.tile([P, n], bf16)             # f.T duplicated in both halves
            GT2 = bigpool.tile([P, n], bf16)             # g.T duplicated in both halves
            haug = bigpool.tile([P, n_tiles, D + 1], bf16)
            nc.gpsimd.memset(haug[:, :, D:D + 1], 1.0)

            IDEN = mybir.ActivationFunctionType.Identity

            def emit_proj_block(ni):
                """Two f/g chains ([Wf|Wg] and [Wg|Wf]) + four h chains for
                x-tiles 4ni..4ni+3.

                Chained PSUM accumulation exposes the PE drain latency, so
                independent chains are interleaved (ring depth 3) to keep
                back-to-back matmuls on different PSUM banks.
                """
                sl = slice(ni * 512, (ni + 1) * 512)
                fgA = psA.tile([P, 512], f32, tag="sp", name=f"fgA{ni}")
                fgB = psA.tile([P, 512], f32, tag="sp", name=f"fgB{ni}")
                hp = [psA.tile([P, D], f32, tag="sp", name=f"hps{4 * ni + t}")
                      for t in range(4)]

                def fg_mm(b, cc):
                    nc.tensor.matmul(
                        fgA if b == 0 else fgB,
                        lhsT=wfg_sb[:, cc, 2 * D * b:2 * D * (b + 1)],
                        rhs=xT[:, cc, sl],
                        start=(cc == 0), stop=(cc == CC - 1),
                    )

                def h_mm(t, cc):
                    i = 4 * ni + t
                    nc.tensor.matmul(
                        hp[t], lhsT=xT[:, cc, i * P:(i + 1) * P],
                        rhs=wh_sb[:, cc, :],
                        start=(cc == 0), stop=(h_bias_zero and cc == CC - 1),
                    )

                # group 1: fgA + fgB + h0 round-robin (3 live PSUM tiles)
                for cc in range(CC):
                    fg_mm(0, cc)
                    fg_mm(1, cc)
                    h_mm(0, cc)
                # A: [f;g] -> FT2 top, GT2 bottom; B: [g;f] -> GT2 top, FT2 bottom
                # (copy+bias+cast split across ScalarE and VectorE -- scalar
                # Copy-class ops pay the 2.3x errata, ~1.3us each)
                nc.vector.tensor_scalar(
                    FT2[0:D, sl], fgA[0:D, :], bfg_sb[0:D, 0:1], None, ADD)
                nc.scalar.activation(GT2[D:P, sl], fgA[D:P, :], IDEN,
                                     bias=bfg_sb[D:P, 0:1], scale=1.0)
                nc.vector.tensor_scalar(
                    GT2[0:D, sl], fgB[0:D, :], bfg_sb[0:D, 1:2], None, ADD)
                nc.scalar.activation(FT2[D:P, sl], fgB[D:P, :], IDEN,
                                     bias=bfg_sb[D:P, 1:2], scale=1.0)
                # group 2: h1 + h2 + h3
                for cc in range(CC):
                    h_mm(1, cc)
                    h_mm(2, cc)
                    h_mm(3, cc)
                for t in range(4):
                    i = 4 * ni + t
                    if not h_bias_zero:
                        nc.tensor.matmul(
                            hp[t], lhsT=ones_sb, rhs=bh_sb, start=False, stop=True)
                    nc.vector.tensor_copy(out=haug[:, i, 0:D], in_=hp[t])

            def emit_qk_exp(jc, ip):
                sl = slice(jc * 512, (jc + 1) * 512)
                i0, i1 = 2 * ip, 2 * ip + 1
                sp = psA.tile([P, 1024], f32, tag="sp", name=f"sp{jc}_{ip}")
                nc.tensor.matmul(
                    sp[:, 0:512],
                    lhsT=GT2[0:D, i0 * P:(i0 + 1) * P], rhs=FT2[0:D, sl],
                    start=True, stop=True, tile_position=(0, 0),
                )
                nc.tensor.matmul(
                    sp[:, 512:1024],
                    lhsT=GT2[D:P, i1 * P:(i1 + 1) * P], rhs=FT2[D:P, sl],
                    start=True, stop=True, tile_position=(D, 0),
                )
                ep = epool.tile([P, 1024], bf16, tag="ep", name=f"ep{jc}_{ip}")
                if ip in (SCH_PAIRS_C0 if jc == 0 else SCH_PAIRS):
                    # VectorE Schraudolph exp: one tensor_scalar into bf16 bits
                    nc.vector.tensor_scalar(
                        ep[:].bitcast(i16), sp, SCH_A, SCH_B, MULT, ADD)
                else:
                    nc.scalar.activation(ep, sp, EXP, bias=zbias[:, 0:1], scale=1.0)
                ep_tiles[(jc, ip)] = ep

            def emit_pv(jc, ip):
                i0, i1 = 2 * ip, 2 * ip + 1
                ep = ep_tiles.pop((jc, ip))
                ctx = ctx_tiles[jc]
                nc.tensor.matmul(
                    ctx, lhsT=haug[:, i0, :], rhs=ep[:, 0:512],
                    start=(ip == 0), stop=False,
                )
                nc.tensor.matmul(
                    ctx, lhsT=haug[:, i1, :], rhs=ep[:, 512:1024],
                    start=False, stop=(ip == n_pairs - 1),
                )

            def chunk_epilogue_a(jc):
                # latency part: PSUM evacuation + denominator gather DMAs.
                # Runs early so the small-DMA latency hides under attention.
                ctx = ctx_tiles[jc]
                ct = ctpool.tile([D + 1, 512], bf16, tag="ct", name=f"ct{jc}")
                nc.vector.tensor_copy(out=ct, in_=ctx)
                # gather the 4 subtile denominators: [1, 512] -> [4, 128] -> [128, 4]
                dn4 = smpool.tile([4, P], bf16, tag="dn", name=f"dn{jc}")
                # the last chunk's gather must not queue behind stores: use
                # the (by then idle) scalar HWDGE queue
                eng = nc.scalar if jc == nch - 1 else nc.sync
                for t in range(4):
                    eng.dma_start(dn4[t:t + 1, :], ct[D:D + 1, t * P:(t + 1) * P])
                ct_tiles[jc] = (ct, dn4)

            def epilogue_units(jc):
                """The chunk epilogue as 5 small units, interleaved between
                attention steps so its matmuls/DVE ops never bunch up."""
                ct, dn4 = ct_tiles.pop(jc)
                rc = smpool.tile([P, 4], f32, tag="rc", name=f"rc{jc}")

                def u_denom():
                    dtp = psA.tile([P, 4], bf16, tag="sp", name=f"dtp{jc}")
                    nc.tensor.transpose(dtp, dn4, id_sb[0:4, 0:4])
                    rce = smpool.tile([P, 4], f32, tag="rce", name=f"rce{jc}")
                    nc.vector.tensor_scalar(rce, dtp, 1e-30, None, ADD)
                    nc.vector.reciprocal(rc, rce)
                yield u_denom

                def u_out(t):
                    it = jc * 4 + t
                    tsl = slice(t * P, (t + 1) * P)
                    op = psA.tile([P, C], f32, tag="sp", name=f"op{it}")
                    nc.tensor.matmul(op, lhsT=ct[:, tsl], rhs=wv_sb, start=True, stop=True)
                    osb = opool.tile([P, C], f32, tag="os", name=f"osb{it}")
                    nc.vector.scalar_tensor_tensor(
                        osb, op, rc[:, t:t + 1], xres[:, it, :], MULT, ADD)
                    if jc == nch - 1:
                        # final chunk's stores are the kernel tail: split across
                        # two DMAs to halve the drain latency
                        nc.sync.dma_start(o_t[it][0:D, :], osb[0:D, :])
                        nc.scalar.dma_start(o_t[it][D:P, :], osb[D:P, :])
                    else:
                        nc.sync.dma_start(o_t[it], osb)
                for t in range(4):
                    yield (lambda tt: (lambda: u_out(tt)))(t)

            # ---- software-pipelined emission over all (chunk, pair) steps:
            # QK+exp run LOOKAHEAD pairs ahead of PV so the PE queue never
            # stalls waiting for an exp; epilogues trail by EPI_DELAY pairs.
            ctx_tiles = {}
            for jc in range(nch):
                ctx_tiles[jc] = psB.tile([D + 1, 512], f32, tag="ctx", name=f"ctx{jc}")
            ep_tiles = {}
            ct_tiles = {}
            pairs = [(jc, ip) for jc in range(nch) for ip in range(n_pairs)]
            LOOKAHEAD = 2
            EPI_DELAY_A = 1
            EPI_DELAY = 10
            n_total = len(pairs)
            qk_done = 0
            pv_done = 0
            epia_done = 0
            epi_done = 0

            xres_done = 0
            epi_queue = []

            def advance_pv():
                nonlocal pv_done, epia_done, epi_done, xres_done
                emit_pv(*pairs[pv_done])
                pv_done += 1
                # paced residual loads: issued as attention progresses so
                # they never crowd out the critical xT loads on sync
                if xres_done < nch and pv_done >= 16 + 2 * xres_done:
                    nc.sync.dma_start(
                        xres[:, 4 * xres_done:4 * xres_done + 4, :],
                        x_blk[xres_done])
                    xres_done += 1
                while epia_done < nch and pv_done >= min(
                        (epia_done + 1) * n_pairs + EPI_DELAY_A, n_total):
                    chunk_epilogue_a(epia_done)
                    epia_done += 1
                while epi_done < nch and pv_done >= min(
                        (epi_done + 1) * n_pairs + EPI_DELAY, n_total):
                    epi_queue.extend(epilogue_units(epi_done))
                    epi_done += 1

            # prologue: per 512-row block, load + project, then advance the
            # attention pipeline through chunk 0's pairs as they become legal
            wv_sb = id_sb = None
            for ni in range(nch):
                nsl = slice(ni * 512, (ni + 1) * 512)
                # xT loads own the sync queue: they gate f/g/h and chunk 0
                nc.sync.dma_start(xT[:, :, nsl], xbt_blk[:, :, nsl])
                if ni == 1:
                    # epilogue-only constants: loaded after the critical path
                    wv_sb = cpool.tile([D + 1, C], bf16)
                    nc.sync.dma_start(wv_sb, wv_d[:, :])
                    id_sb = cpool.tile([P, P], bf16)
                    nc.sync.dma_start(id_sb, id_d[:, :])
                emit_proj_block(ni)
                while qk_done <= 2 * ni + 1:
                    emit_qk_exp(*pairs[qk_done])
                    qk_done += 1
                while pv_done < qk_done - LOOKAHEAD:
                    advance_pv()

            # steady state: advance two pairs per step -- batching the two
            # QKs then the two PVs halves the QK<->PV row-group transitions
            # (each costs ~100ns of LDWEIGHTS that can't overlap in-flight
            # matmuls on conflicting PE rows)
            while pv_done < n_total:
                for _ in range(2):
                    if qk_done < n_total:
                        emit_qk_exp(*pairs[qk_done])
                        qk_done += 1
                while pv_done < min(qk_done - LOOKAHEAD, n_total):
                    advance_pv()
                # one epilogue unit per step keeps its matmuls spread out
                if epi_queue:
                    epi_queue.pop(0)()
                if qk_done == n_total:
                    while pv_done < n_total:
                        advance_pv()
            while epi_queue:
                epi_queue.pop(0)()

    nc.compile()
    return nc


def get_program(n: int = N_FULL, h_bias_zero: bool = False):
    key = (n, h_bias_zero)
    if key not in _CACHE:
        _CACHE[key] = _build(n, h_bias_zero)
    return _CACHE[key]


def make_weight_maps(Wf, bf, Wg, bg, Wh, bh, Wv, bv, gamma, h_bias_zero=False):
    """Host-side layout prep of the tiny replicated weights."""
    wv_aug = np.concatenate(
        [np.float32(gamma) * np.asarray(Wv, np.float32),
         np.asarray(bv, np.float32)[None, :]], axis=0)
    bf = np.asarray(bf, np.float32)
    bg = np.asarray(bg, np.float32)
    bfg = np.stack([np.concatenate([bf, bg]), np.concatenate([bg, bf])], axis=1)
    Wf = np.asarray(Wf, np.float32)
    Wg = np.asarray(Wg, np.float32)
    wfg = np.concatenate([Wf, Wg, Wg, Wf], axis=1)
    maps = {
        "wfg": np.ascontiguousarray(wfg.astype(BF16)),
        "wh": np.ascontiguousarray(np.asarray(Wh, np.float32).astype(BF16)),
        "bfg": np.ascontiguousarray(bfg),
        "bhp": np.ascontiguousarray(np.asarray(bh, np.float32).astype(BF16).reshape(1, D)),
        "onesp": np.ones((1, P), dtype=BF16),
        "wv": np.ascontiguousarray(wv_aug.astype(BF16)),
        "ident": np.ascontiguousarray(np.eye(P, dtype=BF16)),
    }
    if h_bias_zero:
        del maps["bhp"], maps["onesp"]
    return maps


def kernel(x, Wf, bf, Wg, bg, Wh, bh, Wv, bv, gamma):
    from concourse.bass_utils import run_bass_kernel_spmd

    x = np.asarray(x, np.float32)
    b, hh, ww, c = x.shape
    n = hh * ww
    assert (b, c) == (B, C)

    hbz = bool(np.all(np.asarray(bh) == 0))
    nc = get_program(n, hbz)
    base = make_weight_maps(Wf, bf, Wg, bg, Wh, bh, Wv, bv, gamma, hbz)
    xf = x.reshape(b, n, c)
    in_maps = [
        dict(base,
             x=np.ascontiguousarray(xf[i]),
             xbt=np.ascontiguousarray(xf[i].T.astype(BF16)))
        for i in range(b)
    ]

    res = run_bass_kernel_spmd(nc, in_maps, core_ids=list(range(b)))
    out = np.stack([res.results[i]["out"] for i in range(b)], axis=0)
    return np.ascontiguousarray(out.reshape(b, hh, ww, c).astype(np.float32))


# revision 50
# speedup vs baseline: 1.0284x; 1.0132x over previous
"""SAGAN self-attention block on 8 TRN2 NeuronCores.

Reference (per batch element b, N = H*W = 4096, C = 512, D = 64):
    f = x @ Wf + bf ; g = x @ Wg + bg ; h = x @ Wh + bh      # [N, D]
    s = f @ g.T                                              # [N, N]
    attn = softmax(s, axis=-1)
    ctx = attn @ h                                           # [N, D]
    o = (gamma * ctx) @ Wv + bv + x                          # [N, C]

Sharding: data-parallel over batch B=8 -> one batch element per core, no
collectives. Weights replicated.

Device algorithm (per core), matmuls in bf16 with f32 PSUM accumulation:
  - x arrives twice: f32 (residual) and as a host-side bf16 transposed
    copy xbt [C, N] (input layout prep, like the weight packing) -- no
    on-device transposes of x at all, and few, large, contiguous DMAs.
  - f and g projected in one matmul chain per 512-chunk using stacked
    [Wf|Wg] weights (M=128); each half is mirrored into the other
    partition half of FT2/GT2 via SBUF->SBUF DMA so QK pairs row-pack.
  - haug[m, :] = [x@Wh + bh, 1.0] -> [4096, 65] bf16 (m on partitions).
  - unnormalized softmax (no max subtraction: max s ~ 67, e^67 fits bf16):
      for each n-chunk of 512 columns:
        for each pair (i0, i1) of 128-row m-tiles:
          S'[i0]|S'[i1] computed concurrently via K=64 row-packing into
          one [128, 1024] f32 PSUM tile
          E' = exp(S') -> bf16, produced by ONE of:
            * ScalarE ACT exp (exact), or
            * VectorE Schraudolph bit-trick: bf16_bits = i16(s*184.665
              + 16248.5) ~= exp(s) to ~2% rms -- one tensor_scalar op.
          The split keeps both engines busy; softmax ratios stay
          consistent because numerator and denominator use the same E'.
          ctxT[0:65, chunk] += haug[i0].T @ E'[:, :512] + haug[i1].T @ ...
        row 64 of ctxT = sum_m E' = softmax denominator (ones column)
  - epilogue per chunk: denominators for all 4 row-tiles are gathered by
    one tiny DMA + one PE transpose; out = (ctxT.T @ [gamma*Wv ; bv])
    * (1/denom) + x, residual add on GpSimd.
  - prologue (loads, transposes, f/g/h projections) is emitted
    interleaved with attention chunk 0 so exp work starts immediately.
"""

import numpy as np
import ml_dtypes

BF16 = ml_dtypes.bfloat16

B, HH, WW, C = 8, 64, 64, 512
D = C // 8          # 64
N_FULL = HH * WW    # 4096
P = 128
CC = C // P         # 4  (c-chunks of 128)

# Schraudolph bf16 exp: bits = round(x * 184.665 + 16248.5), view as bf16.
SCH_A = 128.0 / float(np.log(2.0))
SCH_B = 16248.5
# Pairs whose exp runs on VectorE via the bit trick (rest on ScalarE).
# Pairs advance in bursts of two (2k, 2k+1); putting the VectorE exp on the
# odd member makes each burst's two exps run on different engines
# concurrently, halving the sp PSUM-slot recycle latency. Chunk 0 runs
# during the prologue, where VectorE is busy with h-copies -- keep its
# VectorE share small and late.
SCH_PAIRS = frozenset({2, 5, 7, 10, 13, 15})
SCH_PAIRS_C0 = SCH_PAIRS

_CACHE: dict = {}


def _build(n: int, h_bias_zero: bool = False):
    """Build + compile the single-core Bass program (same NEFF on all 8 cores)."""
    import concourse.mybir as mybir
    from concourse import bacc
    from concourse.tile import TileContext

    f32 = mybir.dt.float32
    bf16 = mybir.dt.bfloat16
    i16 = mybir.dt.int16
    ADD = mybir.AluOpType.add
    MULT = mybir.AluOpType.mult
    EXP = mybir.ActivationFunctionType.Exp

    n_tiles = n // P          # 32
    n_pairs = n_tiles // 2    # 16
    nch = n // 512            # 8

    nc = bacc.Bacc("TRN2", target_bir_lowering=False, debug=False)

    x_d = nc.dram_tensor("x", [n, C], f32, kind="ExternalInput")
    xbt_d = nc.dram_tensor("xbt", [C, n], bf16, kind="ExternalInput")
    # two stacked projection blocks: [Wf|Wg] and [Wg|Wf] -- the swapped copy
    # lets both partition-halves of FT2/GT2 come straight from matmuls,
    # with no cross-partition mirror DMAs.
    wfg_d = nc.dram_tensor("wfg", [C, 4 * D], bf16, kind="ExternalInput")
    wh_d = nc.dram_tensor("wh", [C, D], bf16, kind="ExternalInput")
    bfg_d = nc.dram_tensor("bfg", [P, 2], f32, kind="ExternalInput")   # [bf;bg],[bg;bf]
    if not h_bias_zero:
        bh_d = nc.dram_tensor("bhp", [1, D], bf16, kind="ExternalInput")
        on_d = nc.dram_tensor("onesp", [1, P], bf16, kind="ExternalInput")
    wv_d = nc.dram_tensor("wv", [D + 1, C], bf16, kind="ExternalInput")
    id_d = nc.dram_tensor("ident", [P, P], bf16, kind="ExternalInput")
    out_d = nc.dram_tensor("out", [n, C], f32, kind="ExternalOutput")

    # big-DMA views: one load per 512-row block
    x_blk = x_d.rearrange("(ni t p) c -> ni p t c", p=P, t=4)
    xbt_blk = xbt_d.rearrange("(cc p) n -> p cc n", p=P)
    o_t = out_d.rearrange("(i p) c -> i p c", p=P)

    with TileContext(nc) as tc:
        with (
            tc.tile_pool(name="const", bufs=1) as cpool,
            tc.tile_pool(name="big", bufs=1) as bigpool,
            tc.tile_pool(name="ep", bufs=6) as epool,
            tc.tile_pool(name="ct", bufs=2) as ctpool,
            tc.tile_pool(name="os", bufs=4) as opool,
            tc.tile_pool(name="sm", bufs=4) as smpool,
            tc.tile_pool(name="psA", bufs=3, space="PSUM") as psA,
            tc.tile_pool(name="psB", bufs=2, space="PSUM") as psB,
        ):
            # ---- replicated constants -> SBUF
            wfg_sb = cpool.tile([P, CC, 4 * D], bf16)
            nc.sync.dma_start(wfg_sb, wfg_d.rearrange("(cc p) d -> p cc d", p=P))
            wh_sb = cpool.tile([P, CC, D], bf16)
            nc.sync.dma_start(wh_sb, wh_d.rearrange("(cc p) d -> p cc d", p=P))
            bfg_sb = cpool.tile([P, 2], f32)
            nc.sync.dma_start(bfg_sb, bfg_d[:, :])
            if not h_bias_zero:
                bh_sb = cpool.tile([1, D], bf16)
                nc.sync.dma_start(bh_sb, bh_d[:, :])
                ones_sb = cpool.tile([1, P], bf16)
                nc.sync.dma_start(ones_sb, on_d[:, :])
            zbias = cpool.tile([P, 1], f32)
            nc.gpsimd.memset(zbias, 0.0)

            # warm the ACT exp table while the prologue DMAs run
            warm = cpool.tile([P, 1], bf16)
            nc.scalar.activation(warm, bfg_sb[:, 0:1], EXP,
                                 bias=zbias[:, 0:1], scale=0.0)

            # ---- persistent SBUF tensors
            xres = bigpool.tile([P, n_tiles, C], f32)    # x rows (residual)
            xT = bigpool.tile([P, CC, n], bf16)          # x transposed (c on partitions)
            FT2 = bigpool.tile([P, n], bf16)             # f.T duplicated in both halves
            GT2 = bigpool.tile([P, n], bf16)             # g.T duplicated in both halves
            haug = bigpool.tile([P, n_tiles, D + 1], bf16)
            nc.gpsimd.memset(haug[:, :, D:D + 1], 1.0)

            IDEN = mybir.ActivationFunctionType.Identity

            def emit_proj_block(ni):
                """Two f/g chains ([Wf|Wg] and [Wg|Wf]) + four h chains for
                x-tiles 4ni..4ni+3.

                Chained PSUM accumulation exposes the PE drain latency, so
                independent chains are interleaved (ring depth 3) to keep
                back-to-back matmuls on different PSUM banks.
                """
                sl = slice(ni * 512, (ni + 1) * 512)
                fgA = psA.tile([P, 512], f32, tag="sp", name=f"fgA{ni}")
                fgB = psA.tile([P, 512], f32, tag="sp", name=f"fgB{ni}")
                hp = [psA.tile([P, D], f32, tag="sp", name=f"hps{4 * ni + t}")
                      for t in range(4)]

                def fg_mm(b, cc):
                    nc.tensor.matmul(
                        fgA if b == 0 else fgB,
                        lhsT=wfg_sb[:, cc, 2 * D * b:2 * D * (b + 1)],
                        rhs=xT[:, cc, sl],
                        start=(cc == 0), stop=(cc == CC - 1),
                    )

                def h_mm(t, cc):
                    i = 4 * ni + t
                    nc.tensor.matmul(
                        hp[t], lhsT=xT[:, cc, i * P:(i + 1) * P],
                        rhs=wh_sb[:, cc, :],
                        start=(cc == 0), stop=(h_bias_zero and cc == CC - 1),
                    )

                # group 1: fgA + fgB + h0 round-robin (3 live PSUM tiles)
                for cc in range(CC):
                    fg_mm(0, cc)
                    fg_mm(1, cc)
                    h_mm(0, cc)
                # A: [f;g] -> FT2 top, GT2 bottom; B: [g;f] -> GT2 top, FT2 bottom
                # (copy+bias+cast split across ScalarE and VectorE -- scalar
                # Copy-class ops pay the 2.3x errata, ~1.3us each)
                nc.vector.tensor_scalar(
                    FT2[0:D, sl], fgA[0:D, :], bfg_sb[0:D, 0:1], None, ADD)
                nc.scalar.activation(GT2[D:P, sl], fgA[D:P, :], IDEN,
                                     bias=bfg_sb[D:P, 0:1], scale=1.0)
                nc.vector.tensor_scalar(
                    GT2[0:D, sl], fgB[0:D, :], bfg_sb[0:D, 1:2], None, ADD)
                nc.scalar.activation(FT2[D:P, sl], fgB[D:P, :], IDEN,
                                     bias=bfg_sb[D:P, 1:2], scale=1.0)
                # group 2: h1 + h2 + h3
                for cc in range(CC):
                    h_mm(1, cc)
                    h_mm(2, cc)
                    h_mm(3, cc)
                for t in range(4):
                    i = 4 * ni + t
                    if not h_bias_zero:
                        nc.tensor.matmul(
                            hp[t], lhsT=ones_sb, rhs=bh_sb, start=False, stop=True)
                    nc.vector.tensor_copy(out=haug[:, i, 0:D], in_=hp[t])

            def emit_qk_exp(jc, ip):
                sl = slice(jc * 512, (jc + 1) * 512)
                i0, i1 = 2 * ip, 2 * ip + 1
                sp = psA.tile([P, 1024], f32, tag="sp", name=f"sp{jc}_{ip}")
                nc.tensor.matmul(
                    sp[:, 0:512],
                    lhsT=GT2[0:D, i0 * P:(i0 + 1) * P], rhs=FT2[0:D, sl],
                    start=True, stop=True, tile_position=(0, 0),
                )
                nc.tensor.matmul(
                    sp[:, 512:1024],
                    lhsT=GT2[D:P, i1 * P:(i1 + 1) * P], rhs=FT2[D:P, sl],
                    start=True, stop=True, tile_position=(D, 0),
                )
                ep = epool.tile([P, 1024], bf16, tag="ep", name=f"ep{jc}_{ip}")
                if ip in (SCH_PAIRS_C0 if jc == 0 else SCH_PAIRS):
                    # VectorE Schraudolph exp: one tensor_scalar into bf16 bits
                    nc.vector.tensor_scalar(
                        ep[:].bitcast(i16), sp, SCH_A, SCH_B, MULT, ADD)
                else:
                    nc.scalar.activation(ep, sp, EXP, bias=zbias[:, 0:1], scale=1.0)
                ep_tiles[(jc, ip)] = ep

            def emit_pv(jc, ip):
                i0, i1 = 2 * ip, 2 * ip + 1
                ep = ep_tiles.pop((jc, ip))
                ctx = ctx_tiles[jc]
                nc.tensor.matmul(
                    ctx, lhsT=haug[:, i0, :], rhs=ep[:, 0:512],
                    start=(ip == 0), stop=False,
                )
                nc.tensor.matmul(
                    ctx, lhsT=haug[:, i1, :], rhs=ep[:, 512:1024],
                    start=False, stop=(ip == n_pairs - 1),
                )

            def chunk_epilogue_a(jc):
                # latency part: PSUM evacuation + denominator gather DMAs.
                # Runs early so the small-DMA latency hides under attention.
                ctx = ctx_tiles[jc]
                ct = ctpool.tile([D + 1, 512], bf16, tag="ct", name=f"ct{jc}")
                nc.vector.tensor_copy(out=ct, in_=ctx)
                # gather the 4 subtile denominators: [1, 512] -> [4, 128] -> [128, 4]
                dn4 = smpool.tile([4, P], bf16, tag="dn", name=f"dn{jc}")
                # the last chunk's gather must not queue behind stores: use
                # the (by then idle) scalar HWDGE queue
                eng = nc.scalar if jc == nch - 1 else nc.sync
                for t in range(4):
                    eng.dma_start(dn4[t:t + 1, :], ct[D:D + 1, t * P:(t + 1) * P])
                ct_tiles[jc] = (ct, dn4)

            def epilogue_units(jc):
                """The chunk epilogue as 5 small units, interleaved between
                attention steps so its matmuls/DVE ops never bunch up."""
                ct, dn4 = ct_tiles.pop(jc)
                rc = smpool.tile([P, 4], f32, tag="rc", name=f"rc{jc}")

                def u_denom():
                    dtp = psA.tile([P, 4], bf16, tag="sp", name=f"dtp{jc}")
                    nc.tensor.transpose(dtp, dn4, id_sb[0:4, 0:4])
                    rce = smpool.tile([P, 4], f32, tag="rce", name=f"rce{jc}")
                    nc.vector.tensor_scalar(rce, dtp, 1e-30, None, ADD)
                    nc.vector.reciprocal(rc, rce)
                yield u_denom

                def u_out(t):
                    it = jc * 4 + t
                    tsl = slice(t * P, (t + 1) * P)
                    op = psA.tile([P, C], f32, tag="sp", name=f"op{it}")
                    nc.tensor.matmul(op, lhsT=ct[:, tsl], rhs=wv_sb, start=True, stop=True)
                    osb = opool.tile([P, C], f32, tag="os", name=f"osb{it}")
                    nc.vector.scalar_tensor_tensor(
                        osb, op, rc[:, t:t + 1], xres[:, it, :], MULT, ADD)
                    if jc == nch - 1:
                        # final chunk's stores are the kernel tail: split across
                        # two DMAs to halve the drain latency
                        nc.sync.dma_start(o_t[it][0:D, :], osb[0:D, :])
                        nc.scalar.dma_start(o_t[it][D:P, :], osb[D:P, :])
                    else:
                        nc.sync.dma_start(o_t[it], osb)
                for t in range(4):
                    yield (lambda tt: (lambda: u_out(tt)))(t)

            # ---- software-pipelined emission over all (chunk, pair) steps:
            # QK+exp run LOOKAHEAD pairs ahead of PV so the PE queue never
            # stalls waiting for an exp; epilogues trail by EPI_DELAY pairs.
            ctx_tiles = {}
            for jc in range(nch):
                ctx_tiles[jc] = psB.tile([D + 1, 512], f32, tag="ctx", name=f"ctx{jc}")
            ep_tiles = {}
            ct_tiles = {}
            pairs = [(jc, ip) for jc in range(nch) for ip in range(n_pairs)]
            LOOKAHEAD = 2
            EPI_DELAY_A = 1
            EPI_DELAY = 10
            n_total = len(pairs)
            qk_done = 0
            pv_done = 0
            epia_done = 0
            epi_done = 0

            xres_done = 0
            epi_queue = []

            def advance_pv():
                nonlocal pv_done, epia_done, epi_done, xres_done
                emit_pv(*pairs[pv_done])
                pv_done += 1
                # paced residual loads: issued as attention progresses so
                # they never crowd out the critical xT loads on sync
                if xres_done < nch and pv_done >= 16 + 2 * xres_done:
                    nc.sync.dma_start(
                        xres[:, 4 * xres_done:4 * xres_done + 4, :],
                        x_blk[xres_done])
                    xres_done += 1
                while epia_done < nch and pv_done >= min(
                        (epia_done + 1) * n_pairs + EPI_DELAY_A, n_total):
                    chunk_epilogue_a(epia_done)
                    epia_done += 1
                while epi_done < nch and pv_done >= min(
                        (epi_done + 1) * n_pairs + EPI_DELAY, n_total):
                    epi_queue.extend(epilogue_units(epi_done))
                    epi_done += 1

            # prologue: per 512-row block, load + project, then advance the
            # attention pipeline through chunk 0's pairs as they become legal
            wv_sb = id_sb = None
            for ni in range(nch):
                nsl = slice(ni * 512, (ni + 1) * 512)
                # xT loads own the sync queue: they gate f/g/h and chunk 0
                nc.sync.dma_start(xT[:, :, nsl], xbt_blk[:, :, nsl])
                if ni == 1:
                    # epilogue-only constants: loaded after the critical path
                    wv_sb = cpool.tile([D + 1, C], bf16)
                    nc.sync.dma_start(wv_sb, wv_d[:, :])
                    id_sb = cpool.tile([P, P], bf16)
                    nc.sync.dma_start(id_sb, id_d[:, :])
                emit_proj_block(ni)
                while qk_done <= 2 * ni + 1:
                    emit_qk_exp(*pairs[qk_done])
                    qk_done += 1
                while pv_done < qk_done - LOOKAHEAD:
                    advance_pv()

            # steady state: advance two pairs per step -- batching the two
            # QKs then the two PVs halves the QK<->PV row-group transitions
            # (each costs ~100ns of LDWEIGHTS that can't overlap in-flight
            # matmuls on conflicting PE rows)
            while pv_done < n_total:
                for _ in range(2):
                    if qk_done < n_total:
                        emit_qk_exp(*pairs[qk_done])
                        qk_done += 1
                while pv_done < min(qk_done - LOOKAHEAD, n_total):
                    advance_pv()
                # one epilogue unit per step keeps its matmuls spread out
                if epi_queue:
                    epi_queue.pop(0)()
                if qk_done == n_total:
                    while pv_done < n_total:
                        advance_pv()
            while epi_queue:
                epi_queue.pop(0)()

    nc.compile()
    return nc


def get_program(n: int = N_FULL, h_bias_zero: bool = False):
    key = (n, h_bias_zero)
    if key not in _CACHE:
        _CACHE[key] = _build(n, h_bias_zero)
    return _CACHE[key]


def make_weight_maps(Wf, bf, Wg, bg, Wh, bh, Wv, bv, gamma, h_bias_zero=False):
    """Host-side layout prep of the tiny replicated weights."""
    wv_aug = np.concatenate(
        [np.float32(gamma) * np.asarray(Wv, np.float32),
         np.asarray(bv, np.float32)[None, :]], axis=0)
    bf = np.asarray(bf, np.float32)
    bg = np.asarray(bg, np.float32)
    bfg = np.stack([np.concatenate([bf, bg]), np.concatenate([bg, bf])], axis=1)
    Wf = np.asarray(Wf, np.float32)
    Wg = np.asarray(Wg, np.float32)
    wfg = np.concatenate([Wf, Wg, Wg, Wf], axis=1)
    maps = {
        "wfg": np.ascontiguousarray(wfg.astype(BF16)),
        "wh": np.ascontiguousarray(np.asarray(Wh, np.float32).astype(BF16)),
        "bfg": np.ascontiguousarray(bfg),
        "bhp": np.ascontiguousarray(np.asarray(bh, np.float32).astype(BF16).reshape(1, D)),
        "onesp": np.ones((1, P), dtype=BF16),
        "wv": np.ascontiguousarray(wv_aug.astype(BF16)),
        "ident": np.ascontiguousarray(np.eye(P, dtype=BF16)),
    }
    if h_bias_zero:
        del maps["bhp"], maps["onesp"]
    return maps


def kernel(x, Wf, bf, Wg, bg, Wh, bh, Wv, bv, gamma):
    from concourse.bass_utils import run_bass_kernel_spmd

    x = np.asarray(x, np.float32)
    b, hh, ww, c = x.shape
    n = hh * ww
    assert (b, c) == (B, C)

    hbz = bool(np.all(np.asarray(bh) == 0))
    nc = get_program(n, hbz)
    base = make_weight_maps(Wf, bf, Wg, bg, Wh, bh, Wv, bv, gamma, hbz)
    xf = x.reshape(b, n, c)
    in_maps = [
        dict(base,
             x=np.ascontiguousarray(xf[i]),
             xbt=np.ascontiguousarray(xf[i].T.astype(BF16)))
        for i in range(b)
    ]

    res = run_bass_kernel_spmd(nc, in_maps, core_ids=list(range(b)))
    out = np.stack([res.results[i]["out"] for i in range(b)], axis=0)
    return np.ascontiguousarray(out.reshape(b, hh, ww, c).astype(np.float32))
